# revision 42
# baseline (speedup 1.0000x reference)
"""MoE block (router + top-2 expert MLPs) on 8 Trainium2 NeuronCores.

Strategy (expert-parallel, fp8 DoubleRow):
  - Router (x @ Wr + br, top-2, softmax) computed on host with jax using the
    exact expression of the reference so expert selection matches bitwise.
  - Tokens are dispatched by expert: core e receives the tokens whose top-2
    includes expert e (padded to a fixed capacity CAP), plus expert e's
    weights W1[e]/b1[e]/W2[e]/b2[e].
  - Each core runs a Bass/Tile kernel computing
        y = sigmoid(relu(x @ W1 + b1) @ W2 + b2)
    for its CAP tokens with fp8-e4m3 matmuls in DoubleRow perf mode
    (2 fp8 weights per PE cell -> K=256 contraction per pass, ~1.4-1.8x
    the fp16 matmul throughput). fp32 PSUM accumulation.
  - Quantization scales (powers of 2, exact to undo): x*16, W1*2048,
    h*32, W2*4096. relu is positively homogeneous so the h scale folds
    into the layer-1 activation (scale=2^-10 on PSUM, bias=32*b1);
    the final sigmoid applies scale=2^-17 to undo h/W2 scaling. b2 is
    pre-scaled by 2^17 on host (fp32/fp16, exact enough).
  - Host combines: out[t] = sum_k weight[t,k] * y_e[t].

Kernel layout per core:
  xT [NGRP, 128, KC, GROUP] fp8 (tokens gathered+transposed+scaled on
  host; one 3 KiB/partition DMA per group),
  W1 [HPAIR, 128, 2, KC, 128] fp8 and W2 [HPAIR, 128, 2, D] fp8 (one
  2 KiB/partition DMA per h-chunk pair; ~250 GB/s sustained),
  b1 fp32 (*32), b2 fp32 (*2^17). All weights are SBUF-resident; they
  stream from HBM exactly once, deadline-ordered: x(g0), first w1
  pairs, then rounds of two w1 pairs + one w2 pair (layer 1 consumes
  w1 about twice as fast as layer 2 consumes w2), w2 tail with
  x(g1)/x(g2)/b2 interleaved.
  Loop over 3 token groups of 384; per group y accumulates in PSUM
  (3 x [128 tok, 1024 d] fp32 tiles = 6 banks) across 16 h-chunk PAIRS;
  the h PSUM tiles (128 h x 384 tok, 2 banks) double-buffer.
  Layer 1 (per h-chunk): 4 DoubleRow matmuls lhsT=W1[:, kc:kc+2, :],
  rhs=xT[:, kc:kc+2, :] -> h^T in PSUM; relu+b1 via ScalarE into a
  [128, 2, GROUP] fp8 pair tile; layer 2: lhsT=pair tile slice
  [128, 2, 128 tok], rhs=W2 pair [128, 2, 512], DoubleRow, accumulating
  into the y PSUM tiles. The layer-1 work for pair j+2 is issued before
  layer-2 of pair j so the PE never stalls on the relu latency.
  Epilogue runs at 512-column half granularity with one PSUM bank per
  (m, h2) half (DVE b2-add, ScalarE sigmoid -> fp16, DMA out) so the
  next group's first accumulation (WAR per half-bank) unblocks as early
  as possible. The last group runs layer 2 m-major (m0 finishes during
  the j loop, then m1/m2 as straight runs) so only m2's sigmoid+store
  trail the final matmul; its b2 is pre-added via rank-1 (K=1) fp16
  matmuls to keep the DVE off the tail.
  Startup: 8 PE warmup matmuls keep the HAM activity window busy while
  the first DMAs land (under-warming extends the 1.2 GHz cold phase);
  a dummy sigmoid preloads the sigmoid-anchored ACT table set during
  the preamble (relu is filler in every set), removing the ~1.3 us
  mid-kernel ACT_TABLE_LOAD from the g0 boundary; the first x/w1
  chunks are triggered on the ACT engine's DGE queue in parallel with
  the Sync queue.
"""

import numpy as np

D = 1024
H = 4096
E = 8
TOPK = 2
B = 4096

P = 128
KC = D // P          # 8 contraction chunks for layer 1
HC = H // P          # 32 h chunks
HPAIR = HC // 2      # 16 h-chunk pairs (DoubleRow)
GROUP = 384          # tokens per PSUM-resident group
MSUB = GROUP // P    # 3 token subtiles per group
NGRP = 3             # groups per core
CAP = GROUP * NGRP   # 1152 token capacity per core
N_CORES = 8

# fp8 quantization scales (powers of two; exactly undone on device)
SX = 16.0
S1 = 2048.0
SH = 32.0
S2 = 4096.0

_compiled_nc = {}


def _build_nc(n_last=GROUP):
    import concourse.bacc as bacc
    import concourse.mybir as mybir
    import concourse.tile as tile

    f32 = mybir.dt.float32
    f16 = mybir.dt.float16
    fp8 = mybir.dt.float8e4
    AF = mybir.ActivationFunctionType
    DR = mybir.MatmulPerfMode.DoubleRow
    DRS = mybir.MatmulPerfMode.DoubleRowSwInterleave

    nc = bacc.Bacc("TRN2", target_bir_lowering=False, debug=False,
                   enable_asserts=False)

    # Host-prearranged layouts: every chunk is one contiguous DMA.
    #   xt[g, p, kc, t'] = SX * x_tokens[g*GROUP + t', kc*128 + p]
    #   w1[j, p, i, kc, h'] = S1 * W1[kc*128 + p, (2j+i)*128 + h']
    #   w2[j, p, i, d] = S2 * W2[(2j+i)*128 + p, d]
    xt_d = nc.dram_tensor("xt", (NGRP, P, KC, GROUP), fp8,
                          kind="ExternalInput")
    # p-major weight layouts: any h-chunk RANGE is a single 2D DMA with
    # per-partition-contiguous lines, so the startup stream can use few
    # large transfers (each dma_start costs ~600 ns of descriptor
    # generation on its sequencer - that, not HBM bandwidth, limits the
    # pipeline fill).
    # w1 stored software-interleaved for DoubleRowSwInterleave: per
    # (h-chunk, k2) the stationary is [P, m-reversed, 2] so LDWEIGHTS
    # reads contiguously (HW DoubleRow's strided interleave gather makes
    # layer 1 - one LDWEIGHTS per matmul - LDWEIGHTS-exposed).
    w1_d = nc.dram_tensor("w1", (P, HC, KC // 2, P, 2), fp8,
                          kind="ExternalInput")
    # b1 pre-transposed on host to [P, HC] so the DMA is one contiguous
    # 128 B line per partition (the [H]-layout gather took ~3.6 us and
    # blocked the weight stream).
    b1_d = nc.dram_tensor("b1", (P, HC), f32, kind="ExternalInput")  # *SH
    w2_d = nc.dram_tensor("w2", (P, HC, D), fp8,
                          kind="ExternalInput")  # *S2
    b2_d = nc.dram_tensor("b2", (D,), f32, kind="ExternalInput")  # *SH*S2
    b2h_d = nc.dram_tensor("b2h", (D,), f16, kind="ExternalInput")  # *SH*S2
    ones_d = nc.dram_tensor("ones", (P,), f16, kind="ExternalInput")
    # Output stored as contiguous [g, m, h2, P, 512] blocks so each store
    # is a plain per-partition-contiguous DMA (cheap ring descriptor, not
    # a ~700 ns DIRECT2D trigger on the sequencer). Host re-interleaves.
    y_d = nc.dram_tensor("y", (NGRP, MSUB, 2, P, 512), f16,
                         kind="ExternalOutput")

    y_v = y_d.ap()

    with tile.TileContext(nc) as tc:
        with (
            tc.tile_pool(name="const", bufs=1) as cpool,
            tc.tile_pool(name="wres", bufs=1) as respool,
            tc.tile_pool(name="hsb", bufs=HPAIR + 2) as hpool,
            tc.tile_pool(name="yout", bufs=4) as ypool_sb,
            tc.tile_pool(name="hps", bufs=2, space="PSUM") as hpsum,
            tc.tile_pool(name="yps", bufs=1, space="PSUM") as ypsum,
        ):
            x_sb = [cpool.tile([P, KC, GROUP], fp8, name=f"x{g}",
                               tag=f"x{g}") for g in range(NGRP)]
            # PE warm-up: dependency-free matmuls on an uninitialized
            # scratch tile get the PE past the HAM half-clock window while
            # the first input DMAs are still in flight. Results land in a
            # scratch PSUM tile and are never read.
            scratch_sb = cpool.tile([P, GROUP], fp8)
            # memset on GpSimd: it runs earliest after the start barrier,
            # so the warmup matmuls (which depend on this write) can begin
            # sooner and start the HAM activity window.
            nc.gpsimd.memset(scratch_sb[:], 0.0)
            ones_sb = cpool.tile([1, P], f16)
            nc.vector.memset(ones_sb[:], 1.0)
            # Just enough warm-up to start the HAM activity window while
            # the first DMAs are in flight; real matmuls then run through
            # the remaining cold phase at half clock doing useful work
            # (cheaper than blocking behind junk warmups for the whole
            # ~3.4 us window).
            warm_ps = hpsum.tile([P, GROUP], f32, name="warm_ps", tag="hps")
            for _ in range(3):
                nc.tensor.matmul(warm_ps[:], scratch_sb[:, :P],
                                 scratch_sb[:], start=True, stop=True)

            w1_all = respool.tile([P, HC, KC // 2, P, 2], fp8)
            w2_all = respool.tile([P, HC, D], fp8)

            def dma_w1r(a, b, eng=None):
                # h-chunk range [a, b) -> one DMA
                (eng or nc.sync).dma_start(w1_all[:, a:b],
                                           w1_d.ap()[:, a:b])

            def dma_w2(j, eng=None):
                (eng or nc.sync).dma_start(w2_all[:, 2 * j:2 * j + 2, :],
                                           w2_d.ap()[:, 2 * j:2 * j + 2])

            # Deadline-ordered input stream, one role per DGE queue.
            # Descriptor generation (~600 ns per dma_start, serial per
            # sequencer) is the startup bottleneck, so: Scalar's queue
            # carries only the latency-critical x(g0) quarters (4 descs,
            # then its sequencer is free for the relu stream); GpSimd's
            # otherwise-idle queue carries the whole w2 stream; Sync
            # carries b1 + the w1 stream as pair-sized ranges (h-chunk
            # units: pair j consumes w1 chunks [2j, 2j+2)) + late extras.
            # Measured budget: ~600 ns descriptor generation per dma_start
            # (serial per sequencer) and only ~2 KB(/partition)/us of HBM
            # supply per core TOTAL across all queues under 8-core load.
            # So: Sync carries b1 + an uninterrupted w1 stream (w1 alone
            # needs ~1 KB/us - half the core's supply); Scalar carries the
            # x(g0) quarters then w2(2,3); GpSimd gets only a small flood
            # (w2(0,1), b2, b2h). Everything else (w2(4..15), x(g1),
            # x(g2)) is trigger-paced from inside the group loops so no
            # queue runs ahead of its deadlines and starves w1.
            nc.scalar.dma_start(x_sb[0][:, :2], xt_d.ap()[0, :, :2])
            dma_w1r(0, 2)
            b1_sb = cpool.tile([P, HC], f32)
            nc.gpsimd.dma_start(b1_sb[:], b1_d.ap())
            nc.scalar.dma_start(x_sb[0][:, 2:4], xt_d.ap()[0, :, 2:4])
            dma_w1r(2, 4, eng=nc.gpsimd)
            nc.scalar.dma_start(x_sb[0][:, 4:6], xt_d.ap()[0, :, 4:6])
            nc.scalar.dma_start(x_sb[0][:, 6:8], xt_d.ap()[0, :, 6:8])
            dma_w1r(4, 6)
            dma_w1r(6, 8, eng=nc.scalar)
            dma_w2(0, eng=nc.gpsimd)
            dma_w1r(8, 10)
            dma_w1r(10, 12, eng=nc.scalar)
            dma_w2(1, eng=nc.gpsimd)
            dma_w1r(12, 14)
            dma_w2(2, eng=nc.scalar)
            for j in range(7, HPAIR):
                dma_w1r(2 * j, 2 * j + 2)
            dma_w2(3, eng=nc.scalar)
            # Dummy sigmoid AFTER the scalar DMA triggers: preloads the
            # sigmoid-anchored ACT table set (relu is filler in every set,
            # so no mid-kernel ACT_TABLE_LOAD), without its ~1.3 us table
            # load blocking Scalar's descriptor generation for x/w1.
            sig_warm = cpool.tile([1, 8], f16)
            nc.scalar.activation(sig_warm[:], scratch_sb[:1, :8], AF.Sigmoid)
            b2_full = cpool.tile([P, D], f32)
            nc.gpsimd.dma_start(
                b2_full[:], b2_d.ap()[None, :].broadcast_to([P, D]))
            b2h_sb = cpool.tile([1, D], f16)
            nc.gpsimd.dma_start(b2h_sb[:], b2h_d.ap()[None, :])

            def layer1_pair(g, j):
                """h^T for h-chunks (2j, 2j+1): DoubleRow matmuls + relu
                into a [P, 2, GROUP] fp8 pair tile. The last group only
                computes n_last columns (the real max token count of the
                critical core); the remaining columns hold stale data that
                layer 2 multiplies into y rows nobody reads."""
                ncols = n_last if g == NGRP - 1 else GROUP
                hsb2 = hpool.tile([P, 2, GROUP], fp8)
                # k2-major, alternating the pair's two h-PSUM banks every
                # matmul: back-to-back accumulation into the same bank
                # pays a read-modify-write turnaround (~20 ns/MM).
                hps = [hpsum.tile([P, GROUP], f32, name=f"hps{i}",
                                  tag="hps") for i in range(2)]
                for k2 in range(KC // 2):
                    for i in range(2):
                        nc.tensor.matmul(
                            hps[i][:, :ncols],
                            w1_all[:, 2 * j + i, k2],
                            x_sb[g][:, 2 * k2:2 * k2 + 2, :ncols],
                            start=(k2 == 0), stop=(k2 == KC // 2 - 1),
                            perf_mode=DRS,
                        )
                # relu(acc/(SX*S1) + b1) * SH, written as
                # relu(acc * SH/(SX*S1) + SH*b1)  (b1 pre-scaled on host)
                for i in range(2):
                    nc.scalar.activation(
                        hsb2[:, i, :ncols], hps[i][:, :ncols], AF.Relu,
                        bias=b1_sb[:, 2 * j + i:2 * j + i + 1],
                        scale=SH / (SX * S1))
                return hsb2

            def layer2_m(g, j, hsb2, yps, m, last):
                # b2 is folded via a rank-1 matmul only for the final
                # m-subtile (the one whose epilogue trails the last
                # matmul); m0/m1 of the last group take the DVE add,
                # which hides behind the next subtile's matmuls.
                fold = last and m == MSUB - 1
                lhs = hsb2[:, :, m * P:(m + 1) * P]
                for h2 in range(2):
                    nc.tensor.matmul(
                        yps[m][h2][:],
                        lhs,
                        w2_all[:, 2 * j:2 * j + 2,
                               h2 * 512:(h2 + 1) * 512],
                        start=(j == 0 and not fold),
                        stop=(j == HPAIR - 1),
                        perf_mode=DR,
                    )

            def layer2_pair(g, j, hsb2, yps, last):
                for m in range(MSUB):
                    layer2_m(g, j, hsb2, yps, m, last)

            for g in range(NGRP):
                # One PSUM tile (= one bank) per (m, h2) half so the
                # epilogue chain (DVE b2-add -> sigmoid -> next group's
                # WAR) resolves per half-bank, not per [P, D] tile.
                yps = [[ypsum.tile([P, 512], f32, name=f"yps{m}h{h2}",
                                   tag=f"yps{m}h{h2}") for h2 in range(2)]
                       for m in range(MSUB)]

                last = g == NGRP - 1

                def fold_b2(m):
                    # rank-1 b2 matmuls keep the last group's tail short
                    # (no DVE add on the critical path)
                    for h2 in range(2):
                        nc.tensor.matmul(
                            yps[m][h2][:],
                            ones_sb[:],
                            b2h_sb[:, h2 * 512:(h2 + 1) * 512],
                            start=True, stop=False,
                        )

                def epilogue(m):
                    # (+ b2 via DVE unless folded), sigmoid -> fp16, store
                    # at 512-column halves. EMISSION position matters: m0
                    # and m1's sigmoids are emitted inside the m2 run and
                    # m2's right after it, so ScalarE does them while it
                    # has no relu backlog (the j loop runs ScalarE near
                    # saturation; injecting sigmoids there stalls layer 1
                    # through the h-PSUM WAR chain).
                    fold = last and m == MSUB - 1
                    for h2 in range(2):
                        if not fold:
                            nc.vector.tensor_add(yps[m][h2][:], yps[m][h2][:],
                                                 b2_full[:, h2 * 512:
                                                         (h2 + 1) * 512])
                        yo = ypool_sb.tile([P, 512], f16)
                        nc.scalar.activation(yo[:], yps[m][h2][:], AF.Sigmoid,
                                             scale=1.0 / (SH * S2))
                        eng = (nc.gpsimd if h2 == 0 else nc.sync) \
                            if fold else nc.sync
                        eng.dma_start(y_v[g, m, h2], yo[:])

                # m-major layer 2 for every group, with m1 lagging two
                # pairs behind m0 inside the j loop (4 layer-2 matmuls per
                # layer-1 pair dilutes the relu density to ~70% ScalarE
                # util), then m2 as a straight run. Each m's epilogue runs
                # during the next m's matmuls, so no group-boundary WAR on
                # the y banks and no sigmoid burst into the next group's
                # relu-saturated j loop. (All pair tiles stay alive:
                # hpool bufs >= HPAIR+2.)
                hq = [layer1_pair(g, 0)]
                hq.append(layer1_pair(g, 1))
                if g == 0:
                    # g0 is input-bandwidth-bound (the whole weight stream
                    # lands during it), so run the gentler interleaved
                    # schedule: all three m-subtiles advance together and
                    # the loop consumes one w1+w2 pair per ~2.8 us instead
                    # of m-major's ~2.2 us. Its 6-sigmoid epilogue burst is
                    # absorbed by g1's m-major j loop (ScalarE ~50% there).
                    for j in range(HPAIR):
                        if j + 2 < HPAIR:
                            hq.append(layer1_pair(g, j + 2))
                            if j + 2 >= 4:
                                # paced w2 supply on Scalar's queue
                                dma_w2(j + 2, eng=nc.scalar)
                        if j == 11:
                            nc.scalar.dma_start(x_sb[1][:], xt_d.ap()[1])
                        layer2_pair(g, j, hq[j], yps, last)
                    for m in range(MSUB):
                        epilogue(m)
                else:
                    # weights are SBUF-resident now; m-major layer 2 with
                    # m1 lagging two pairs inside the j loop, m2 straight,
                    # per-m epilogues during the next m's matmuls: no
                    # group-boundary y-bank WAR, no sigmoid burst into a
                    # relu-saturated phase.
                    for j in range(HPAIR):
                        if j + 2 < HPAIR:
                            hq.append(layer1_pair(g, j + 2))
                        if g == 1 and j == 8:
                            nc.scalar.dma_start(x_sb[2][:], xt_d.ap()[2])
                        layer2_m(g, j, hq[j], yps, 0, last)
                        if j >= 2:
                            layer2_m(g, j - 2, hq[j - 2], yps, 1, last)
                    layer2_m(g, HPAIR - 2, hq[HPAIR - 2], yps, 1, last)
                    layer2_m(g, HPAIR - 1, hq[HPAIR - 1], yps, 1, last)
                    epilogue(0)
                    if last:
                        fold_b2(MSUB - 1)
                    for j in range(HPAIR):
                        layer2_m(g, j, hq[j], yps, 2, last)
                        if j == 3:
                            epilogue(1)
                    epilogue(2)

    nc.compile()
    return nc


def _routing(x, Wr, br):
    """Router computed with the same jax expression as the reference."""
    import jax
    import jax.numpy as jnp

    logits = jnp.asarray(x) @ jnp.asarray(Wr) + jnp.asarray(br)
    topk_vals, topk_idx = jax.lax.top_k(logits, TOPK)
    weights = jax.nn.softmax(topk_vals, axis=-1)
    return np.asarray(topk_idx), np.asarray(weights, np.float32)


def _get_nc(n_last=GROUP):
    if n_last not in _compiled_nc:
        _compiled_nc[n_last] = _build_nc(n_last)
    return _compiled_nc[n_last]


def _to_fp8(a):
    import ml_dtypes
    return a.astype(ml_dtypes.float8_e4m3fn)


def kernel(x, Wr, br, W1, b1, W2, b2, _trace=False, _trace_kwargs=None):
    from concourse import bass_utils

    x = np.ascontiguousarray(np.asarray(x, dtype=np.float32))
    Wr = np.asarray(Wr, dtype=np.float32)
    br = np.asarray(br, dtype=np.float32)
    W1 = np.asarray(W1, dtype=np.float32)
    b1 = np.asarray(b1, dtype=np.float32)
    W2 = np.asarray(W2, dtype=np.float32)
    b2 = np.asarray(b2, dtype=np.float32)

    topk_idx, wts = _routing(x, Wr, br)

    # Per-expert token lists and weights
    tok_lists = []
    wt_lists = []
    for e in range(E):
        mask = topk_idx == e                      # [B, TOPK]
        toks = np.nonzero(mask.any(axis=1))[0]
        # weight of expert e for each selected token (exactly one slot matches)
        slot = mask[toks].argmax(axis=1)
        tok_lists.append(toks)
        wt_lists.append(wts[toks, slot])

    max_count = max(len(t) for t in tok_lists)
    n_waves = max(1, -(-max_count // CAP))
    # Specialize the last group's layer-1 width to the real token count of
    # the critical core (routing is deterministic for a given input).
    if n_waves == 1:
        n_last = min(GROUP, max(8, max_count - 2 * GROUP))
    else:
        n_last = GROUP
    nc = _get_nc(n_last)

    xq = _to_fp8(x * SX)
    # p-major sw-interleaved w1 layout: [P, HC, KC//2, P(m rev), 2],
    # w1sw[p, hc, k2, mr, i] = S1*W1[(2k2+i)*128+p, hc*128+(127-mr)]
    W1ch = [np.ascontiguousarray(
        _to_fp8(W1[e] * S1).reshape(KC // 2, 2, P, HC, P)
        .transpose(2, 3, 0, 4, 1)[:, :, :, ::-1, :]) for e in range(E)]
    # p-major w2 layout: [P, HC, D], scaled by S2
    W2ch = [np.ascontiguousarray(
        _to_fp8(W2[e] * S2).reshape(HC, P, D).transpose(1, 0, 2))
        for e in range(E)]

    out = np.zeros((B, D), dtype=np.float32)
    last_result = None
    for wave in range(n_waves):
        in_maps = []
        for e in range(E):
            toks = tok_lists[e][wave * CAP:(wave + 1) * CAP]
            xpad = np.zeros((CAP, D), dtype=xq.dtype)
            if len(toks):
                xpad[:len(toks)] = xq[toks]
            # [NGRP, P, KC, GROUP]: xt[g, p, kc, t] = xpad[g*384+t, kc*128+p]
            xt = np.ascontiguousarray(
                xpad.reshape(NGRP, GROUP, KC, P).transpose(0, 3, 2, 1))
            in_maps.append({
                "xt": xt,
                "ones": np.ones((P,), dtype=np.float16),
                "b2h": (b2[e] * SH * S2).astype(np.float16),
                "w1": W1ch[e],
                "b1": np.ascontiguousarray((b1[e] * SH).reshape(HC, P).T),
                "w2": W2ch[e],
                "b2": np.ascontiguousarray(b2[e] * SH * S2),
            })
        res = bass_utils.run_bass_kernel_spmd(
            nc, in_maps, core_ids=list(range(N_CORES)),
            trace=_trace, **(_trace_kwargs or {}))
        last_result = res
        for e in range(E):
            toks = tok_lists[e][wave * CAP:(wave + 1) * CAP]
            if len(toks) == 0:
                continue
            y_full = res.results[e]["y"].transpose(0, 1, 3, 2, 4) \
                .reshape(CAP, D)
            y_e = y_full[:len(toks)].astype(np.float32)
            out[toks] += wt_lists[e][wave * CAP:(wave + 1) * CAP][:, None] * y_e

    if _trace:
        kernel.last_result = last_result
    return out



# revision 44
# speedup vs baseline: 1.0369x; 1.0369x over previous
"""MoE block (router + top-2 expert MLPs) on 8 Trainium2 NeuronCores.

Strategy (expert-parallel, fp8 DoubleRow):
  - Router (x @ Wr + br, top-2, softmax) computed on host with jax using the
    exact expression of the reference so expert selection matches bitwise.
  - Tokens are dispatched by expert: core e receives the tokens whose top-2
    includes expert e (padded to a fixed capacity CAP), plus expert e's
    weights W1[e]/b1[e]/W2[e]/b2[e].
  - Each core runs a Bass/Tile kernel computing
        y = sigmoid(relu(x @ W1 + b1) @ W2 + b2)
    for its CAP tokens with fp8-e4m3 DoubleRow matmuls (K=256/pass,
    ~0.42 ns/column warm). fp32 PSUM accumulation.
  - Quantization scales (powers of 2, exact to undo): x*16, W1*2048,
    h*32, W2*4096; sigmoid applies 2^-17; b2 pre-scaled by 2^17 on host.
  - Host combines: out[t] = sum_k weight[t,k] * y_e[t].

Kernel structure per core (3 groups of 384 tokens; last group's layer-1
width specialized to the real max token count):
  Layer 1 per h-chunk pair: k2-major DoubleRowSwInterleave matmuls
  (w1 host-pre-interleaved [P, m-rev, 2] per (hc, k2)), alternating the
  pair's two h-PSUM banks; relu+b1 via ScalarE into fp8 pair tiles.
  Layer 2: lhsT = h pair slice [128, 2, 128 tok], rhs = W2 pair
  [128, 2, 512], accumulating into 6 one-bank y PSUM tiles [128, 512].
  Layer-1 for pair j+2 issues before layer-2 of pair j (relu hiding).
  The last group runs layer 2 m-major (m0 during the j loop, m1/m2 as
  straight runs) so only m2's sigmoid+store trail the final matmul; its
  b2 is folded via rank-1 matmuls emitted after the j loop (waits out
  the previous group's m2-sigmoid WAR off the PE critical path).
  Epilogue per 512-column half: DVE b2-add (except folded m2), ScalarE
  sigmoid -> fp16, store; the final stores ride the GpSimd/Sync queues
  so their triggers don't serialize behind ScalarE.

Timing facts this schedule is built on (measured):
  - PE warm at 2.4 GHz: ~0.42 ns/col; HAM cold phase ~3.4 us at 1.2 GHz
    from first PE activity (3 warmups start the window during the DMA
    wait; occasionally the chip drops to ~2.0 GHz under sustained load).
  - Each dma_start costs ~600 ns of descriptor generation, serialized
    per sequencer; HBM supply is ~2 KB(/partition)/us per core total
    under 8-core load. Hence: x(g0) quarters + a few w1/w2 chunks on the
    Scalar queue (before its relu stream begins), the rest of the
    deadline-ordered w1/w2 stream on Sync, w1 running several pairs
    ahead (p-major layouts make any h-chunk range a single DMA).
  - The dummy sigmoid that preloads the sigmoid ACT table is emitted
    after the DMA triggers (its ~1.3 us table load otherwise blocks
    Scalar's descriptor generation); the warmup scratch memset runs on
    GpSimd so the warmups start as early as possible.
  - Output y is stored as contiguous [g, m, h2, 128, 512] blocks (host
    re-interleaves): strided stores cost ~700 ns DIRECT2D descriptor
    generations on the Sync sequencer, which inflated every matmul's
    issue path.
"""

import numpy as np

D = 1024
H = 4096
E = 8
TOPK = 2
B = 4096

P = 128
KC = D // P          # 8 contraction chunks for layer 1
HC = H // P          # 32 h chunks
HPAIR = HC // 2      # 16 h-chunk pairs (DoubleRow)
GROUP = 384          # tokens per PSUM-resident group
MSUB = GROUP // P    # 3 token subtiles per group
NGRP = 3             # groups per core
CAP = GROUP * NGRP   # 1152 token capacity per core
N_CORES = 8

# fp8 quantization scales (powers of two; exactly undone on device)
SX = 16.0
S1 = 2048.0
SH = 32.0
S2 = 4096.0

_compiled_nc = {}


def _build_nc(n_last=GROUP):
    import concourse.bacc as bacc
    import concourse.mybir as mybir
    import concourse.tile as tile

    f32 = mybir.dt.float32
    f16 = mybir.dt.float16
    fp8 = mybir.dt.float8e4
    AF = mybir.ActivationFunctionType
    DR = mybir.MatmulPerfMode.DoubleRow
    DRS = mybir.MatmulPerfMode.DoubleRowSwInterleave

    nc = bacc.Bacc("TRN2", target_bir_lowering=False, debug=False,
                   enable_asserts=False)

    # Host-prearranged layouts: every chunk is one contiguous DMA.
    #   xt[g, p, kc, t'] = SX * x_tokens[g*GROUP + t', kc*128 + p]
    #   w1[j, p, i, kc, h'] = S1 * W1[kc*128 + p, (2j+i)*128 + h']
    #   w2[j, p, i, d] = S2 * W2[(2j+i)*128 + p, d]
    xt_d = nc.dram_tensor("xt", (NGRP, P, KC, GROUP), fp8,
                          kind="ExternalInput")
    # p-major weight layouts: any h-chunk RANGE is a single 2D DMA with
    # per-partition-contiguous lines, so the startup stream can use few
    # large transfers (each dma_start costs ~600 ns of descriptor
    # generation on its sequencer - that, not HBM bandwidth, limits the
    # pipeline fill).
    # w1 stored software-interleaved for DoubleRowSwInterleave: per
    # (h-chunk, k2) the stationary is [P, m-reversed, 2] so LDWEIGHTS
    # reads contiguously (HW DoubleRow's strided interleave gather makes
    # layer 1 - one LDWEIGHTS per matmul - LDWEIGHTS-exposed).
    w1_d = nc.dram_tensor("w1", (P, HC, KC // 2, P, 2), fp8,
                          kind="ExternalInput")
    # b1 pre-transposed on host to [P, HC] so the DMA is one contiguous
    # 128 B line per partition (the [H]-layout gather took ~3.6 us and
    # blocked the weight stream).
    b1_d = nc.dram_tensor("b1", (P, HC), f32, kind="ExternalInput")  # *SH
    w2_d = nc.dram_tensor("w2", (P, HC, D), fp8,
                          kind="ExternalInput")  # *S2
    b2_d = nc.dram_tensor("b2", (D,), f32, kind="ExternalInput")  # *SH*S2
    b2h_d = nc.dram_tensor("b2h", (D,), f16, kind="ExternalInput")  # *SH*S2
    ones_d = nc.dram_tensor("ones", (P,), f16, kind="ExternalInput")
    # Output stored as contiguous [g, m, h2, P, 512] blocks so each store
    # is a plain per-partition-contiguous DMA (cheap ring descriptor, not
    # a ~700 ns DIRECT2D trigger on the sequencer). Host re-interleaves.
    y_d = nc.dram_tensor("y", (NGRP, MSUB, 2, P, 512), f16,
                         kind="ExternalOutput")

    y_v = y_d.ap()

    with tile.TileContext(nc) as tc:
        with (
            tc.tile_pool(name="const", bufs=1) as cpool,
            tc.tile_pool(name="wres", bufs=1) as respool,
            tc.tile_pool(name="hsb", bufs=HPAIR + 2) as hpool,
            tc.tile_pool(name="yout", bufs=4) as ypool_sb,
            tc.tile_pool(name="hps", bufs=2, space="PSUM") as hpsum,
            tc.tile_pool(name="yps", bufs=1, space="PSUM") as ypsum,
        ):
            x_sb = [cpool.tile([P, KC, GROUP], fp8, name=f"x{g}",
                               tag=f"x{g}") for g in range(NGRP)]
            # PE warm-up: dependency-free matmuls on an uninitialized
            # scratch tile get the PE past the HAM half-clock window while
            # the first input DMAs are still in flight. Results land in a
            # scratch PSUM tile and are never read.
            scratch_sb = cpool.tile([P, GROUP], fp8)
            # memset on GpSimd: it runs earliest after the start barrier,
            # so the warmup matmuls (which depend on this write) can begin
            # sooner and start the HAM activity window.
            nc.gpsimd.memset(scratch_sb[:], 0.0)
            ones_sb = cpool.tile([1, P], f16)
            nc.vector.memset(ones_sb[:], 1.0)
            # Just enough warm-up to start the HAM activity window while
            # the first DMAs are in flight; real matmuls then run through
            # the remaining cold phase at half clock doing useful work
            # (cheaper than blocking behind junk warmups for the whole
            # ~3.4 us window).
            warm_ps = hpsum.tile([P, GROUP], f32, name="warm_ps", tag="hps")
            for _ in range(3):
                nc.tensor.matmul(warm_ps[:], scratch_sb[:, :P],
                                 scratch_sb[:], start=True, stop=True)

            w1_all = respool.tile([P, HC, KC // 2, P, 2], fp8)
            w2_all = respool.tile([P, HC, D], fp8)

            def dma_w1r(a, b, eng=None):
                # h-chunk range [a, b) -> one DMA
                (eng or nc.sync).dma_start(w1_all[:, a:b],
                                           w1_d.ap()[:, a:b])

            def dma_w2(j, eng=None):
                (eng or nc.sync).dma_start(w2_all[:, 2 * j:2 * j + 2, :],
                                           w2_d.ap()[:, 2 * j:2 * j + 2])

            # Deadline-ordered input stream, one role per DGE queue.
            # Descriptor generation (~600 ns per dma_start, serial per
            # sequencer) is the startup bottleneck, so: Scalar's queue
            # carries only the latency-critical x(g0) quarters (4 descs,
            # then its sequencer is free for the relu stream); GpSimd's
            # otherwise-idle queue carries the whole w2 stream; Sync
            # carries b1 + the w1 stream as pair-sized ranges (h-chunk
            # units: pair j consumes w1 chunks [2j, 2j+2)) + late extras.
            # Measured budget: ~600 ns descriptor generation per dma_start
            # (serial per sequencer) and only ~2 KB(/partition)/us of HBM
            # supply per core TOTAL across all queues under 8-core load.
            # So: Sync carries b1 + an uninterrupted w1 stream (w1 alone
            # needs ~1 KB/us - half the core's supply); Scalar carries the
            # x(g0) quarters then w2(2,3); GpSimd gets only a small flood
            # (w2(0,1), b2, b2h). Everything else (w2(4..15), x(g1),
            # x(g2)) is trigger-paced from inside the group loops so no
            # queue runs ahead of its deadlines and starves w1.
            nc.scalar.dma_start(x_sb[0][:, :2], xt_d.ap()[0, :, :2])
            b1_sb = cpool.tile([P, HC], f32)
            nc.sync.dma_start(b1_sb[:], b1_d.ap())
            nc.scalar.dma_start(x_sb[0][:, 2:4], xt_d.ap()[0, :, 2:4])
            dma_w1r(0, 2)
            nc.scalar.dma_start(x_sb[0][:, 4:6], xt_d.ap()[0, :, 4:6])
            dma_w1r(2, 4)
            nc.scalar.dma_start(x_sb[0][:, 6:8], xt_d.ap()[0, :, 6:8])
            dma_w1r(4, 6)
            dma_w1r(6, 8, eng=nc.scalar)
            dma_w1r(8, 12)
            dma_w2(0, eng=nc.scalar)
            dma_w1r(12, 16)
            dma_w2(1, eng=nc.scalar)
            dma_w2(2)
            dma_w2(3)
            for i in range(7):
                dma_w1r(16 + 2 * i, 18 + 2 * i)
                dma_w2(4 + i)
            dma_w1r(30, 32)
            dma_w2(11)
            dma_w2(12)
            dma_w2(13)
            nc.sync.dma_start(x_sb[1][:], xt_d.ap()[1])
            dma_w2(14)
            b2_full = cpool.tile([P, D], f32)
            nc.sync.dma_start(
                b2_full[:], b2_d.ap()[None, :].broadcast_to([P, D]))
            nc.sync.dma_start(x_sb[2][:], xt_d.ap()[2])
            dma_w2(15)
            b2h_sb = cpool.tile([1, D], f16)
            nc.sync.dma_start(b2h_sb[:], b2h_d.ap()[None, :])
            # Dummy sigmoid AFTER the scalar DMA triggers: preloads the
            # sigmoid-anchored ACT table set (relu is filler in every set,
            # so no mid-kernel ACT_TABLE_LOAD), without its ~1.3 us table
            # load blocking Scalar's descriptor generation for x/w1.
            sig_warm = cpool.tile([1, 8], f16)
            nc.scalar.activation(sig_warm[:], scratch_sb[:1, :8], AF.Sigmoid)

            def layer1_pair(g, j):
                """h^T for h-chunks (2j, 2j+1): DoubleRow matmuls + relu
                into a [P, 2, GROUP] fp8 pair tile. The last group only
                computes n_last columns (the real max token count of the
                critical core); the remaining columns hold stale data that
                layer 2 multiplies into y rows nobody reads."""
                ncols = n_last if g == NGRP - 1 else GROUP
                hsb2 = hpool.tile([P, 2, GROUP], fp8)
                # k2-major, alternating the pair's two h-PSUM banks every
                # matmul: back-to-back accumulation into the same bank
                # pays a read-modify-write turnaround (~20 ns/MM).
                hps = [hpsum.tile([P, GROUP], f32, name=f"hps{i}",
                                  tag="hps") for i in range(2)]
                for k2 in range(KC // 2):
                    for i in range(2):
                        nc.tensor.matmul(
                            hps[i][:, :ncols],
                            w1_all[:, 2 * j + i, k2],
                            x_sb[g][:, 2 * k2:2 * k2 + 2, :ncols],
                            start=(k2 == 0), stop=(k2 == KC // 2 - 1),
                            perf_mode=DRS,
                        )
                # relu(acc/(SX*S1) + b1) * SH, written as
                # relu(acc * SH/(SX*S1) + SH*b1)  (b1 pre-scaled on host)
                for i in range(2):
                    nc.scalar.activation(
                        hsb2[:, i, :ncols], hps[i][:, :ncols], AF.Relu,
                        bias=b1_sb[:, 2 * j + i:2 * j + i + 1],
                        scale=SH / (SX * S1))
                return hsb2

            def layer2_m(g, j, hsb2, yps, m, last):
                # b2 is folded via a rank-1 matmul only for the final
                # m-subtile (the one whose epilogue trails the last
                # matmul); m0/m1 of the last group take the DVE add,
                # which hides behind the next subtile's matmuls.
                fold = last and m == MSUB - 1
                lhs = hsb2[:, :, m * P:(m + 1) * P]
                for h2 in range(2):
                    nc.tensor.matmul(
                        yps[m][h2][:],
                        lhs,
                        w2_all[:, 2 * j:2 * j + 2,
                               h2 * 512:(h2 + 1) * 512],
                        start=(j == 0 and not fold),
                        stop=(j == HPAIR - 1),
                        perf_mode=DR,
                    )

            def layer2_pair(g, j, hsb2, yps, last):
                for m in range(MSUB):
                    layer2_m(g, j, hsb2, yps, m, last)

            for g in range(NGRP):
                # One PSUM tile (= one bank) per (m, h2) half so the
                # epilogue chain (DVE b2-add -> sigmoid -> next group's
                # WAR) resolves per half-bank, not per [P, D] tile.
                yps = [[ypsum.tile([P, 512], f32, name=f"yps{m}h{h2}",
                                   tag=f"yps{m}h{h2}") for h2 in range(2)]
                       for m in range(MSUB)]

                last = g == NGRP - 1

                def fold_b2(m):
                    # rank-1 b2 matmuls keep the last group's tail short
                    # (no DVE add on the critical path)
                    for h2 in range(2):
                        nc.tensor.matmul(
                            yps[m][h2][:],
                            ones_sb[:],
                            b2h_sb[:, h2 * 512:(h2 + 1) * 512],
                            start=True, stop=False,
                        )

                def epilogue(m):
                    # (+ b2 via DVE unless folded), sigmoid -> fp16, store
                    # at 512-column halves. EMISSION position matters: m0
                    # and m1's sigmoids are emitted inside the m2 run and
                    # m2's right after it, so ScalarE does them while it
                    # has no relu backlog (the j loop runs ScalarE near
                    # saturation; injecting sigmoids there stalls layer 1
                    # through the h-PSUM WAR chain).
                    fold = last and m == MSUB - 1
                    for h2 in range(2):
                        if not fold:
                            nc.vector.tensor_add(yps[m][h2][:], yps[m][h2][:],
                                                 b2_full[:, h2 * 512:
                                                         (h2 + 1) * 512])
                        yo = ypool_sb.tile([P, 512], f16)
                        nc.scalar.activation(yo[:], yps[m][h2][:], AF.Sigmoid,
                                             scale=1.0 / (SH * S2))
                        eng = (nc.gpsimd if h2 == 0 else nc.sync) \
                            if fold else nc.sync
                        eng.dma_start(y_v[g, m, h2], yo[:])

                # m-major layer 2 for every group, with m1 lagging two
                # pairs behind m0 inside the j loop (4 layer-2 matmuls per
                # layer-1 pair dilutes the relu density to ~70% ScalarE
                # util), then m2 as a straight run. Each m's epilogue runs
                # during the next m's matmuls, so no group-boundary WAR on
                # the y banks and no sigmoid burst into the next group's
                # relu-saturated j loop. (All pair tiles stay alive:
                # hpool bufs >= HPAIR+2.)
                hq = [layer1_pair(g, 0)]
                hq.append(layer1_pair(g, 1))
                if not last:
                    for j in range(HPAIR):
                        if j + 2 < HPAIR:
                            hq.append(layer1_pair(g, j + 2))
                        layer2_pair(g, j, hq[j], yps, last)
                else:
                    # m-major layer 2: finish m0's accumulation during the
                    # j loop, then m1 and m2 as straight runs, so only m2's
                    # sigmoid+store remain after the very last matmul.
                    for j in range(HPAIR):
                        if j + 2 < HPAIR:
                            hq.append(layer1_pair(g, j + 2))
                        layer2_m(g, j, hq[j], yps, 0, last)
                    # fold after the j loop: the previous group's m2
                    # sigmoids (WAR on the bank) have retired by now, and
                    # m2's first accumulation is still a full m-run away
                    fold_b2(MSUB - 1)
                    for m in (1, 2):
                        for j in range(HPAIR):
                            layer2_m(g, j, hq[j], yps, m, last)

                for m in range(MSUB):
                    epilogue(m)

    nc.compile()
    return nc


def _routing(x, Wr, br):
    """Router computed with the same jax expression as the reference."""
    import jax
    import jax.numpy as jnp

    logits = jnp.asarray(x) @ jnp.asarray(Wr) + jnp.asarray(br)
    topk_vals, topk_idx = jax.lax.top_k(logits, TOPK)
    weights = jax.nn.softmax(topk_vals, axis=-1)
    return np.asarray(topk_idx), np.asarray(weights, np.float32)


def _get_nc(n_last=GROUP):
    if n_last not in _compiled_nc:
        _compiled_nc[n_last] = _build_nc(n_last)
    return _compiled_nc[n_last]


def _to_fp8(a):
    import ml_dtypes
    return a.astype(ml_dtypes.float8_e4m3fn)


def kernel(x, Wr, br, W1, b1, W2, b2, _trace=False, _trace_kwargs=None):
    from concourse import bass_utils

    x = np.ascontiguousarray(np.asarray(x, dtype=np.float32))
    Wr = np.asarray(Wr, dtype=np.float32)
    br = np.asarray(br, dtype=np.float32)
    W1 = np.asarray(W1, dtype=np.float32)
    b1 = np.asarray(b1, dtype=np.float32)
    W2 = np.asarray(W2, dtype=np.float32)
    b2 = np.asarray(b2, dtype=np.float32)

    topk_idx, wts = _routing(x, Wr, br)

    # Per-expert token lists and weights
    tok_lists = []
    wt_lists = []
    for e in range(E):
        mask = topk_idx == e                      # [B, TOPK]
        toks = np.nonzero(mask.any(axis=1))[0]
        # weight of expert e for each selected token (exactly one slot matches)
        slot = mask[toks].argmax(axis=1)
        tok_lists.append(toks)
        wt_lists.append(wts[toks, slot])

    max_count = max(len(t) for t in tok_lists)
    n_waves = max(1, -(-max_count // CAP))
    # Specialize the last group's layer-1 width to the real token count of
    # the critical core (routing is deterministic for a given input).
    if n_waves == 1:
        n_last = min(GROUP, max(8, max_count - 2 * GROUP))
    else:
        n_last = GROUP
    nc = _get_nc(n_last)

    xq = _to_fp8(x * SX)
    # p-major sw-interleaved w1 layout: [P, HC, KC//2, P(m rev), 2],
    # w1sw[p, hc, k2, mr, i] = S1*W1[(2k2+i)*128+p, hc*128+(127-mr)]
    W1ch = [np.ascontiguousarray(
        _to_fp8(W1[e] * S1).reshape(KC // 2, 2, P, HC, P)
        .transpose(2, 3, 0, 4, 1)[:, :, :, ::-1, :]) for e in range(E)]
    # p-major w2 layout: [P, HC, D], scaled by S2
    W2ch = [np.ascontiguousarray(
        _to_fp8(W2[e] * S2).reshape(HC, P, D).transpose(1, 0, 2))
        for e in range(E)]

    out = np.zeros((B, D), dtype=np.float32)
    last_result = None
    for wave in range(n_waves):
        in_maps = []
        for e in range(E):
            toks = tok_lists[e][wave * CAP:(wave + 1) * CAP]
            xpad = np.zeros((CAP, D), dtype=xq.dtype)
            if len(toks):
                xpad[:len(toks)] = xq[toks]
            # [NGRP, P, KC, GROUP]: xt[g, p, kc, t] = xpad[g*384+t, kc*128+p]
            xt = np.ascontiguousarray(
                xpad.reshape(NGRP, GROUP, KC, P).transpose(0, 3, 2, 1))
            in_maps.append({
                "xt": xt,
                "ones": np.ones((P,), dtype=np.float16),
                "b2h": (b2[e] * SH * S2).astype(np.float16),
                "w1": W1ch[e],
                "b1": np.ascontiguousarray((b1[e] * SH).reshape(HC, P).T),
                "w2": W2ch[e],
                "b2": np.ascontiguousarray(b2[e] * SH * S2),
            })
        res = bass_utils.run_bass_kernel_spmd(
            nc, in_maps, core_ids=list(range(N_CORES)),
            trace=_trace, **(_trace_kwargs or {}))
        last_result = res
        for e in range(E):
            toks = tok_lists[e][wave * CAP:(wave + 1) * CAP]
            if len(toks) == 0:
                continue
            y_full = res.results[e]["y"].transpose(0, 1, 3, 2, 4) \
                .reshape(CAP, D)
            y_e = y_full[:len(toks)].astype(np.float32)
            out[toks] += wt_lists[e][wave * CAP:(wave + 1) * CAP][:, None] * y_e

    if _trace:
        kernel.last_result = last_result
    return out

